# revision 19
# baseline (speedup 1.0000x reference)
"""Trainium2 Bass kernel for nn_LASCC (sparse patch-correlation attention + top-k).

Math (per batch element b):
  x_hat = L2-normalize(x, dim=channels)
  z_p[c, n] = x_hat at the two in-patch diagonal pixels (p=0: (0,0), p=1: (1,1))
  C_p = z_p^T z_p                  (1024x1024 normalized correlation, symmetric)
  C_2 = (C_0 + C_1)/2              (avg map)
  s_q = alpha * mask * C_q
  A_q = softmax_row(s_q) * softmax_col(s_q); s symmetric => A = exp(2 a t) u_n u_m
  out pixel with patch n, map q: top-3 over m of A_q[n, m]

Slabs store t_q: t_0 = mask*C_0, t_1 = mask*C_1, t_2 = t_0 + t_1 (so
q=2 needs NO matmuls and no mask pass: a_2 = alpha/2 instead of alpha).

Log-domain top-k: order over m of A[n, m] == order of T[n, m] = t[n, m]
+ ln(u_m)/a2_q (a2_q = 2 a_q), so the F-phase is ONE fp16 2x tensor-add
+ max8; the top-3 VALUES are recovered with a tiny exp on [128, 8, 3]:
out = exp(a2_q * T_top3) * u_n.  One full-size exp per chunk remains
(row sums), with accum_out giving the row sums for free.

ln(u_m) is needed along the FREE (column) dim but is computed in row
layout [128, 8].  Since s is symmetric, col sums == row sums, and the
broadcast lnubc[p, 128*i+n] = lnsc[n, i] is exactly 8 PE matmuls with
lhsT = lnsc[:, i] stride-0-replicated along M and rhs = identity:
out[m, n] = sum_k lnsc[k, i] * I[k, n] = lnsc[n, i].  No DRAM round
trip, no transpose.  The broadcast matmuls + PSUM->SBUF copy for stage
k are emitted early in stage k+1's chunk loop so the PE queue never
head-of-line blocks on them.

Schedule: phase N for batch 0 runs first; batch 1's normalize chains
are injected into stage (0,0)'s chunk loop (they only gate stage 3).
F(k-1) pieces (T-add, max8 x8, tail) are interleaved into the back
half of stage k's chunk loop, so only F(last) drains at the end.
GpSimd offload of T-adds/slab-adds was measured net-negative (sem
coalescing couples the DVE queue to GpSimd completions, and its
software add runs ~4-5x slower per element), so N_GP_* default to 0.
"""
import numpy as np

import concourse.bass as bass
import concourse.mybir as mybir
from concourse import bacc
from concourse.tile import TileContext
from concourse.bass_utils import run_bass_kernel_spmd

F32 = mybir.dt.float32
FP16 = mybir.dt.float16
AF = mybir.ActivationFunctionType
ALU = mybir.AluOpType

B_FULL = 16
N_CORES = 8
B_LOC = B_FULL // N_CORES  # 2
C = 128
H = W = 64
NPH = 32
NP = 1024
PS = 2
TOPK = 3
NCHUNK = NP // 128  # 8
BAND = 512

N_GP_TADD = 0   # chunks (of 4) per T-add half that run on GpSimd
N_GP_S2 = 0     # chunks (of 4) per q2 slab-add group that run on GpSimd

LAST_EXEC_NS = None


def _band_c0(i: int) -> int:
    return min(max(128 * i - 192, 0), NP - BAND)


def _free_bcast_ap(tile_ap, free_dims):
    ap = tile_ap
    new = [ap.ap[0]] + [list(d) for d in free_dims]
    return bass.AP(ap.tensor, ap.offset, new)


def build_nc():
    import concourse.bacc as _bacc_mod
    _orig_tables = _bacc_mod.get_activation_tables

    def _one_table(arch):
        t = _orig_tables(arch)
        # keep dict order (act_func_set_id = index) but leave only the
        # ln+exp superset populated so every activation shares one table
        return {k: (v if k == "natural_log_exp_and_others" else set())
                for k, v in t.items()}

    _bacc_mod.get_activation_tables = _one_table
    try:
        return _build_nc_inner()
    finally:
        _bacc_mod.get_activation_tables = _orig_tables


def _build_nc_inner():
    nc = bacc.Bacc(trn_type="TRN2")

    x_d = nc.dram_tensor("x", [B_LOC, C, H * W], F32, kind="ExternalInput")
    alpha_d = nc.dram_tensor("alpha", [128, 1], F32, kind="ExternalInput")
    mask_d = nc.dram_tensor("mask", [128, NCHUNK, BAND], FP16,
                            kind="ExternalInput")
    ident_d = nc.dram_tensor("ident", [128, 128], FP16, kind="ExternalInput")
    out_d = nc.dram_tensor("out", [B_LOC, 3, 128, NCHUNK, TOPK], F32,
                           kind="ExternalOutput")

    with TileContext(nc) as tc:
        with tc.tile_pool(name="const", bufs=1) as cpool, \
             tc.tile_pool(name="zp", bufs=1) as zpool, \
             tc.tile_pool(name="slab", bufs=2) as slabp, \
             tc.tile_pool(name="ssl", bufs=4) as sslp, \
             tc.tile_pool(name="work", bufs=3) as work, \
             tc.tile_pool(name="wsc", bufs=3) as wscp, \
             tc.tile_pool(name="small", bufs=3) as small, \
             tc.tile_pool(name="ps", bufs=2, space="PSUM") as ps, \
             tc.tile_pool(name="psn", bufs=2, space="PSUM") as psn:

            # ---- input DMAs (xs0 first: it gates the critical path)
            xs_t = {}
            for b in range(B_LOC):
                xs_t[b] = slabp.tile([128, H * W], F32, name=f"xs{b}",
                                     tag="xs", bufs=2)
            for h in range(2):  # chunked halves
                nc.sync.dma_start(xs_t[0][:, 2048 * h:2048 * (h + 1)],
                                  x_d[0][:, 2048 * h:2048 * (h + 1)])

            ones_k = cpool.tile([128, 1], FP16)   # colsum matmul lhsT
            nc.vector.memset(ones_k, 1.0)
            ones_r = cpool.tile([1, 128], FP16)   # K=1 bcast matmul lhsT
            nc.vector.memset(ones_r, 1.0)
            av = cpool.tile([128, 1], F32)        # alpha
            nc.sync.dma_start(av, alpha_d[:, :])
            ident = cpool.tile([128, 128], FP16)
            nc.sync.dma_start(ident, ident_d[:, :])
            av_h = cpool.tile([128, 1], F32)      # alpha/2
            nc.vector.tensor_scalar_mul(av_h, av, 0.5)
            av_d = cpool.tile([128, 1], F32)      # 2*alpha
            nc.vector.tensor_scalar_mul(av_d, av, 2.0)
            rav2n = cpool.tile([128, 1], F32)     # -1/(2*alpha)
            nc.vector.reciprocal(rav2n, av_d)
            nc.vector.tensor_scalar_mul(rav2n, rav2n, -1.0)
            rav1n = cpool.tile([128, 1], F32)     # -1/alpha
            nc.vector.reciprocal(rav1n, av)
            nc.vector.tensor_scalar_mul(rav1n, rav1n, -1.0)
            scale_E = [av, av, av_h]        # a_q for the rowsum exp
            scale_T = [av_d, av_d, av]      # 2 a_q for the tiny value exp
            scale_Ln = [rav2n, rav2n, rav1n]  # -1/(2 a_q): ln(u) from ln(R)

            # ---- mask band (fp16, [p, i, j] with j a 512 window per chunk)
            mask_sb = cpool.tile([128, NCHUNK, BAND], FP16)
            nc.sync.dma_start(mask_sb, mask_d[:, :, :])

            for h in range(2):
                nc.sync.dma_start(xs_t[1][:, 2048 * h:2048 * (h + 1)],
                                  x_d[1][:, 2048 * h:2048 * (h + 1)])

            # ---- phase N pieces (emitted per batch; b=1 injected later)
            nrms = {}
            lnns = {}
            inv = {}
            zp = {}

            def xview(b):
                return xs_t[b].rearrange("c (i r j s) -> c r s i j",
                                         r=PS, s=PS, j=NPH)

            def n_zsq(b, p):
                zv = xview(b)[:, p, p]
                zsq = work.tile([128, NP], FP16, name="zsq", tag="zsq",
                                bufs=2)
                zsr = zsq.rearrange("c (a b) -> c a b", a=NPH)
                for h in range(2):  # halves pipeline with the x DMA chunks
                    nc.vector.tensor_tensor(
                        out=zsr[:, 16 * h:16 * (h + 1)],
                        in0=zv[:, 16 * h:16 * (h + 1)],
                        in1=zv[:, 16 * h:16 * (h + 1)], op=ALU.mult)
                nrm = psn.tile([1, NP], F32, name="nrm", tag="nrm", bufs=1)
                for h in range(2):
                    nc.tensor.matmul(nrm[:, 512 * h:512 * (h + 1)], ones_k,
                                     zsq[:, 512 * h:512 * (h + 1)],
                                     start=True, stop=True)
                nrms[(b, p)] = nrm

            def n_ln(b, p):
                lnn = small.tile([1, NP], F32, name="lnn", tag="lnn", bufs=2)
                nc.scalar.activation(lnn, nrms[(b, p)], AF.Ln)
                lnns[(b, p)] = lnn

            def n_inv(b, p):  # inv = exp(-0.5 ln nrm2)
                inv1 = small.tile([1, NP], FP16, name="inv1", tag="inv1",
                                  bufs=2)
                nc.scalar.activation(inv1, lnns[(b, p)], AF.Exp, scale=-0.5)
                inv[(b, p)] = inv1

            def n_z(b, p):
                ibc = psn.tile([128, NP], F32, name="ibc", tag="ibc", bufs=1)
                for h in range(2):
                    nc.tensor.matmul(ibc[:, 512 * h:512 * (h + 1)], ones_r,
                                     inv[(b, p)][:, 512 * h:512 * (h + 1)],
                                     start=True, stop=True)
                z = zpool.tile([128, NP], FP16, name=f"z{b}{p}",
                               tag=f"z{b}{p}", bufs=1)
                nc.vector.tensor_tensor(
                    out=z.rearrange("c (a b) -> c a b", a=NPH),
                    in0=xview(b)[:, p, p],
                    in1=ibc.rearrange("c (a b) -> c a b", a=NPH), op=ALU.mult)
                zp[(b, p)] = z

            # batch 0 chains now (critical path to first stage)
            for p in range(PS):
                n_zsq(0, p)
            for p in range(PS):
                n_ln(0, p)
                n_inv(0, p)
            for p in range(PS):
                n_z(0, p)

            # batch 1 chain pieces, injected into stage (0,0)'s chunk loop
            b1_sched = {
                (0, 4): lambda: n_zsq(1, 0),
                (0, 5): lambda: n_zsq(1, 1),
                (0, 6): lambda: (n_ln(1, 0), n_inv(1, 0), n_z(1, 0)),
                (0, 7): lambda: (n_ln(1, 1), n_inv(1, 1), n_z(1, 1)),
            }

            # ---- phase M: per-chunk interleaved software pipeline
            s_of = {}  # (b, q) -> slab

            def E_chunk(b, q, i, s_sl, rT, sgp2=None):
                if q < 2:
                    zs = zp[(b, q)]
                    G = ps.tile([128, NP], F32, name="G", tag="G")
                    for h in range(2):
                        nc.tensor.matmul(
                            G[:, 512 * h:512 * (h + 1)],
                            zs[:, 128 * i:128 * (i + 1)],
                            zs[:, 512 * h:512 * (h + 1)],
                            start=True, stop=True)
                    # mask==1 outside a 512-wide band around the diagonal
                    # blocks: multiply only the band (DVE), copy the rest
                    c0 = _band_c0(i)
                    nc.vector.tensor_tensor(
                        out=s_sl[:, i, c0:c0 + BAND],
                        in0=G[:, c0:c0 + BAND],
                        in1=mask_sb[:, i, :], op=ALU.mult)
                    for a, bnd in ((0, c0), (c0 + BAND, NP)):
                        if bnd > a:
                            nc.scalar.copy(s_sl[:, i, a:bnd], G[:, a:bnd])
                else:
                    s0, s1 = s_of[(b, 0)], s_of[(b, 1)]
                    if i % 4 == 0:  # batched 4-chunk add, split DVE/GpSimd
                        nd = 4 - N_GP_S2
                        if N_GP_S2:  # gpsimd first: it is the slow engine
                            nc.gpsimd.tensor_tensor(
                                out=sgp2[:, i // 4, :, :],
                                in0=s0[:, i + nd:i + 4, :],
                                in1=s1[:, i + nd:i + 4, :], op=ALU.add)
                        nc.vector.tensor_tensor(
                            out=s_sl[:, i:i + nd, :], in0=s0[:, i:i + nd, :],
                            in1=s1[:, i:i + nd, :], op=ALU.add)
                e_scr = work.tile([128, NP], FP16, name="e_scr", tag="e_scr",
                                  bufs=2)
                src = (s_sl[:, i, :] if (q < 2 or i % 4 < 4 - N_GP_S2)
                       else sgp2[:, i // 4, i % 4 - (4 - N_GP_S2), :])
                nc.scalar.activation(e_scr, src, AF.Exp,
                                     scale=scale_E[q],
                                     accum_out=rT[:, i:i + 1])

            def E_tail_sums(b, q, s_sl, rT):
                u8 = small.tile([128, NCHUNK], F32, name="u8", tag="u8",
                                bufs=3)
                nc.vector.reciprocal(u8, rT)
                lnr = small.tile([128, NCHUNK], F32, name="lnr", tag="lnr")
                nc.scalar.activation(lnr, rT, AF.Ln)
                lnsc = small.tile([128, NCHUNK], FP16, name="lnsc",
                                  tag="lnsc", bufs=2)
                nc.vector.tensor_scalar_mul(lnsc, lnr, scale_Ln[q][:, 0:1])
                return dict(s_sl=s_sl, lnsc=lnsc, u8=u8, b=b, q=q)

            def E_tail_bcast(stg):
                # lnubc[p, 128*i + n] = lnsc[n, i] via 8 identity matmuls
                lnps = psn.tile([128, NP], F32, name="lnps", tag="ibc",
                                bufs=1)
                for i in range(NCHUNK):
                    lhsT = _free_bcast_ap(stg["lnsc"][:, i:i + 1], [[0, 128]])
                    nc.tensor.matmul(lnps[:, 128 * i:128 * (i + 1)],
                                     lhsT, ident, start=True, stop=True)
                lnubc = wscp.tile([128, NP], FP16, name="lnubc", tag="lnubc",
                                  bufs=2)
                nc.scalar.copy(lnubc, lnps)
                stg["lnubc"] = lnubc

            def F_thalf(stg, h, t8):
                # batched T-add over 4 chunks: T = s + ln(u_m)/(2 a_q)
                # gpsimd owns the tail chunks in its own tile (separate
                # tile: per-tile dep tracking would false-couple max8s)
                ngp = stg["ngp"]
                nd = 4 - ngp
                T_bat = wscp.tile([128, nd, NP], FP16, name=f"T_bat{h}",
                                  tag=f"T_bat{h}", bufs=2)
                T_gp = None
                if ngp:
                    T_gp = wscp.tile([128, ngp, NP], FP16,
                                     name=f"T_gp{h}", tag=f"T_gp{h}", bufs=2)
                    lnb2 = _free_bcast_ap(stg["lnubc"][:, :],
                                          [[0, ngp], [1, NP]])
                    gsrc = stg.get("sgp2")
                    if gsrc is not None:  # q2: gpsimd chunks live in sgp2
                        in0 = gsrc[:, h, :, :]
                    else:
                        in0 = stg["s_sl"][:, 4 * h + nd:4 * (h + 1), :]
                    nc.gpsimd.tensor_tensor(out=T_gp, in0=in0, in1=lnb2,
                                            op=ALU.add)
                lnb = _free_bcast_ap(stg["lnubc"][:, :], [[0, nd], [1, NP]])
                nc.vector.tensor_tensor(
                    out=T_bat, in0=stg["s_sl"][:, 4 * h:4 * h + nd, :],
                    in1=lnb, op=ALU.add)
                return (T_bat, T_gp)

            def F_max8(stg, i, Tpair, t8):
                T_bat, T_gp = Tpair
                j = i % 4
                nd = 4 - stg["ngp"]
                if j < nd:
                    nc.vector.max(out=t8[:, i, :], in_=T_bat[:, j, :])
                else:
                    nc.vector.max(out=t8[:, i, :], in_=T_gp[:, j - nd, :])

            def F_tail(stg, t8):
                b, q = stg["b"], stg["q"]
                tex = small.tile([128, NCHUNK, TOPK], F32, name="tex",
                                 tag="tex")
                nc.scalar.activation(tex, t8[:, :, :TOPK], AF.Exp,
                                     scale=scale_T[q])
                oacc = small.tile([128, NCHUNK, TOPK], F32, name="oacc",
                                  tag="oacc")
                u8b = _free_bcast_ap(stg["u8"][:, :], [[1, NCHUNK], [0, TOPK]])
                nc.vector.tensor_tensor(out=oacc, in0=tex, in1=u8b,
                                        op=ALU.mult)
                # [p, i, k] layout: contiguous DRAM run per partition
                # (the (i p)-major layout costs 1024 12-byte descriptors)
                nc.sync.dma_start(out_d[b, q], oacc)

            def F_pieces(stg, t8):
                """Generator of F-phase emission pieces for one stage."""
                holder = {}

                def thalf(h):
                    def run():
                        holder[h] = F_thalf(stg, h, t8)
                    return run

                def m8(j):
                    def run():
                        F_max8(stg, j, holder[j // 4], t8)
                    return run

                nd = 4 - stg["ngp"]
                order = [j for j in range(4) if j % 4 < nd] + \
                        [j for j in range(4) if j % 4 >= nd]
                yield thalf(0)
                for j in order:
                    yield m8(j)
                yield thalf(1)
                for j in order:
                    yield m8(j + 4)
                yield (lambda: F_tail(stg, t8))

            # per-chunk slots for F(k-1) pieces within stage k's loop
            F_SLOT = {3: 2, 4: 2, 5: 2, 6: 2, 7: 3}

            stages = [(b, q) for b in range(B_LOC) for q in range(3)]
            pend = []     # stages awaiting lnubc broadcast emission
            prevF = None  # piece iterator of F(k-1)
            for k, (b, q) in enumerate(stages):
                s_sl = sslp.tile([128, NCHUNK, NP], FP16, name="s_sl",
                                 tag="s_sl")
                sgp2 = None
                if q == 2 and N_GP_S2:
                    sgp2 = wscp.tile([128, 2, N_GP_S2, NP], FP16,
                                     name="sgp2", tag="sgp2", bufs=2)
                s_of[(b, q)] = s_sl
                rT = small.tile([128, NCHUNK], F32, name="rT", tag="rT")
                for i in range(NCHUNK):
                    E_chunk(b, q, i, s_sl, rT, sgp2)
                    if i == 1 and pend:
                        E_tail_bcast(pend.pop(0))
                    bp = b1_sched.get((k, i))
                    if bp is not None:
                        bp()
                    if prevF is not None:
                        for _ in range(F_SLOT.get(i, 0)):
                            piece = next(prevF, None)
                            if piece is None:
                                break
                            piece()
                if prevF is not None:  # drain leftovers (shouldn't happen)
                    for piece in prevF:
                        piece()
                stg = E_tail_sums(b, q, s_sl, rT)
                stg["sgp2"] = sgp2
                stg["ngp"] = (N_GP_S2 if q == 2 else N_GP_TADD)
                pend.append(stg)
                t8 = small.tile([128, NCHUNK, 8], FP16, name="t8", tag="t8",
                                bufs=3)
                prevF = F_pieces(stg, t8)
            for stg in pend:
                E_tail_bcast(stg)
            for piece in prevF:
                piece()

    nc.compile()
    return nc


_NC_CACHE = None


def _get_nc():
    global _NC_CACHE
    if _NC_CACHE is None:
        _NC_CACHE = build_nc()
    return _NC_CACHE


def _build_mask_band() -> np.ndarray:
    rat_s = np.float32(0.05)
    sr = np.float32(NPH) * rat_s
    ind_r = np.arange(NPH, dtype=np.float32).reshape(1, NPH, 1)
    ind_c = np.arange(NPH, dtype=np.float32).reshape(1, 1, NPH)
    cent = np.arange(NPH, dtype=np.float32)
    cent_r = np.repeat(cent, NPH).reshape(NP, 1, 1)
    cent_c = np.tile(cent, NPH).reshape(NP, 1, 1)
    g = np.exp(-((ind_r - cent_r) ** 2) / (2.0 * sr * sr)) * np.exp(
        -((ind_c - cent_c) ** 2) / (2.0 * sr * sr)
    )
    full = (1.0 - g).reshape(NP, NP).astype(np.float16)
    band = np.empty((128, NCHUNK, BAND), dtype=np.float16)
    for i in range(NCHUNK):
        c0 = _band_c0(i)
        band[:, i, :] = full[128 * i:128 * (i + 1), c0:c0 + BAND]
    return band


def kernel(x: np.ndarray, alpha: np.ndarray) -> np.ndarray:
    global LAST_EXEC_NS
    x = np.ascontiguousarray(np.asarray(x, dtype=np.float32))
    alpha_arr = np.full((128, 1), np.float32(np.asarray(alpha)),
                        dtype=np.float32)
    mask = _build_mask_band()
    ident = np.eye(128, dtype=np.float16)

    nc = _get_nc()
    in_maps = []
    for core in range(N_CORES):
        xs = x[core * B_LOC:(core + 1) * B_LOC].reshape(B_LOC, C, H * W)
        in_maps.append({"x": np.ascontiguousarray(xs), "alpha": alpha_arr,
                        "mask": mask, "ident": ident})
    res = run_bass_kernel_spmd(nc, in_maps, core_ids=list(range(N_CORES)))
    LAST_EXEC_NS = res.exec_time_ns

    out = np.empty((B_FULL, TOPK, H, W), dtype=np.float32)
    for core in range(N_CORES):
        t = res.results[core]["out"]
        for bl in range(B_LOC):
            bg = core * B_LOC + bl
            tr = t[bl].transpose(0, 2, 1, 3).reshape(3, NP, TOPK)
            tq = tr.reshape(3, NPH, NPH, TOPK).transpose(0, 3, 1, 2)
            out[bg, :, 0::2, 0::2] = tq[0]
            out[bg, :, 1::2, 1::2] = tq[1]
            out[bg, :, 0::2, 1::2] = tq[2]
            out[bg, :, 1::2, 0::2] = tq[2]
    return out
